# revision 54
# baseline (speedup 1.0000x reference)
"""Trainium2 Bass kernel for nn_BlendHydroV1 (HBV + ExpHydro blend + gamma routing).

v2 strategy
-----------
Shard 4000 basins over 8 cores (500/core, x NMUL=2 -> 1024 padded columns laid
out [128 partitions, 8 lanes]). Main 730-step loop carries only the five
coupled states (sp, mw, sm, suz-perc, s1) with a minimal fused-op schedule
(~34 DVE/Pool ops + 3 ACT per step after interval-analysis dead-code
elimination):

  * snow: 9 ops via MELT = relu(sp - relu(sp+S-RU)) identity
  * soil: 12 ops + Ln/Exp (rech clip dead since sm<=fc; BLNFC shift)
  * upper zone: 6 ops via suz' = min(a*r - perc, a(1-k0)*r + c - perc)
  * exphydro s1: 7 ops + Exp (ET collapses to S1C*(1-pet/smax) host plane)

Everything else (q0/q1/q2/qspill/qb recovery, pa, slz scan, blend, routing)
runs post-loop as large-free-dim ops from three stored state sequences
(RE, SUZP, S1N) plus precomputed IN.
"""
import numpy as np

S, G, NMUL, LENF = 730, 4000, 2, 15
NCORES = 8
GPC = G // NCORES            # basins per core (500)
BPAD = 512                   # padded basins per core
NCOL = BPAD * NMUL           # 1024 columns
NPART = 128
NL = NCOL // NPART           # 8 lanes per partition
U = 73                       # time steps per For_i iteration
NITER = S // U               # 10
NEARZERO = 1e-5
QSPAD = LENF - 1             # 14
QSW = S + QSPAD              # 744
SEQ = S * NL                 # 5840
SEQ1 = (S + 1) * NL

f32 = np.float32
HBV_LB = np.array([1., 50., .05, .01, .001, .2, 0., 0., -2.5, .5, 0., 0.], f32)
HBV_UB = np.array([6., 1000., .9, .5, .2, 1., 10., 100., 2.5, 10., .1, .2], f32)
EXP_LB = np.array([0., 100., 10., 0., 0., -3.], f32)
EXP_UB = np.array([.1, 1500., 50., 5., 3., 0.], f32)

# param lane order in the `par` DRAM tensor (each NL wide)
PAR_NAMES = ["cwh", "beta", "blnfc", "fc", "ifc", "a1", "k0c", "cp2",
             "perc", "smax", "nfe", "fs", "qmx", "k2c", "hsm"]
NPARAM = len(PAR_NAMES)      # 15
W4_OFF = NPARAM * NL         # 120
PAR_W = W4_OFF + LENF * 4    # 180

# in-loop forcing plane order (slot ((t*NPL)+f)*NL + j)
PLANES = ["d0", "rv", "rain", "pet", "pic", "ipc"]
NPL = len(PLANES)


# --------------------------------------------------------------------------
# host-side preparation
# --------------------------------------------------------------------------

def _host_prepare(x, raw_phy_static):
    """Build per-core DRAM arrays. Returns list of dicts (one per core)."""
    x = np.ascontiguousarray(np.asarray(x, f32))
    raw = np.ascontiguousarray(np.asarray(raw_phy_static, f32))

    static = raw[:, :18 * NMUL].reshape(G, 18, NMUL)
    ph = (HBV_LB[None, :, None] + static[:, :12, :]
          * (HBV_UB - HBV_LB)[None, :, None]).astype(f32)
    pe = (EXP_LB[None, :, None] + static[:, 12:, :]
          * (EXP_UB - EXP_LB)[None, :, None]).astype(f32)

    def cols(a):      # [G, NMUL] -> [G*NMUL], col = g*2+m
        return np.ascontiguousarray(a).reshape(-1)

    beta, fc, k0, k1, k2, lp, perc, uzl, tt, cfmax, cfr, cwh = \
        [cols(ph[:, i, :]) for i in range(12)]
    fexp, smax, qmax, df, tmax, tmin = [cols(pe[:, i, :]) for i in range(6)]

    a1 = (f32(1) - k1).astype(f32)
    params = dict(
        cwh=cwh,
        beta=beta,
        blnfc=(beta * np.log(fc)).astype(f32),
        fc=fc,
        ifc=(1.0 / fc.astype(np.float64)).astype(f32),
        a1=a1,
        k0c=(f32(1) - k0).astype(f32),
        # upper zone runs in sm/fc-scaled units: perc and c are pre-divided
        cp2=(((a1 * k0 * uzl - perc).astype(np.float64)
              / fc.astype(np.float64)).astype(f32)),
        perc=(perc.astype(np.float64) / fc.astype(np.float64)).astype(f32),
        smax=smax,
        nfe=fexp.astype(f32),
        fs=(fexp * smax).astype(f32),
        qmx=qmax,
        k2c=(f32(1) - k2).astype(f32),
        hsm=(f32(0.5) * smax).astype(f32),
    )

    P = x[:, :, 0]
    T = x[:, :, 1]
    PET = x[:, :, 2]
    Pc = np.repeat(P, NMUL, axis=1)
    Tc = np.repeat(T, NMUL, axis=1)
    PETc = np.repeat(PET, NMUL, axis=1)

    rain = np.where(Tc >= tt[None, :], Pc, f32(0)).astype(f32)
    sn = (Pc - rain).astype(f32)
    ru = np.maximum((cfmax[None, :] * (Tc - tt[None, :])).astype(f32), f32(0))
    rv = np.maximum(((cfr * cfmax)[None, :] * (tt[None, :] - Tc)).astype(f32),
                    f32(0))
    d0 = (sn - ru).astype(f32)
    pic = (f32(1) - PETc * (1.0 / smax.astype(np.float64)).astype(f32)[None, :]
           ).astype(f32)
    ilpfc_v = (1.0 / (lp.astype(np.float64) * fc.astype(np.float64))).astype(f32)
    ipc = (f32(1) - PETc * ilpfc_v[None, :]).astype(f32)

    ps = np.where(Tc <= tmin[None, :], Pc, f32(0)).astype(f32)
    mc = np.where(Tc > tmax[None, :],
                  (df[None, :] * (Tc - tmax[None, :])).astype(f32), f32(0))
    scan_c = (ps - mc).astype(f32)

    # routing weights (reference _uh_gamma in f32, scaled by 0.25)
    from scipy.special import gammaln
    route = raw[:, 18 * NMUL:]
    a = (route[:, 0] * f32(2.9)).astype(f32)
    b = (route[:, 1] * f32(6.5)).astype(f32)
    aa = (np.maximum(a, f32(0)) + f32(0.1)).astype(f32)
    th = (np.maximum(b, f32(0)) + f32(0.5)).astype(f32)
    tgrid = (np.arange(LENF, dtype=f32) + f32(0.5))
    logw = (-gammaln(aa.astype(np.float64)).astype(f32)[None, :]
            - (aa * np.log(th).astype(f32))[None, :]
            + np.outer(np.log(tgrid).astype(f32), (aa - f32(1)))
            - np.outer(tgrid, (1.0 / th.astype(np.float64)).astype(f32)))
    w = np.exp(logw.astype(f32)).astype(f32)
    w = (w / w.sum(0, keepdims=True)).astype(f32)          # [LENF, G]
    w4 = (w * f32(0.25)).astype(f32)

    ifc_v = (1.0 / fc.astype(np.float64)).astype(f32)
    peti = (PETc * ifc_v[None, :]).astype(f32)
    nrv = (-rv).astype(f32)
    planes = [d0, nrv, rain, peti, pic, ipc]  # in-loop forcing order
    pre_planes = [scan_c, ps, Pc]           # C, B, PTOT (j-major per core)

    per_core = []
    for d in range(NCORES):
        c0, c1 = d * GPC * NMUL, (d + 1) * GPC * NMUL     # 1000 cols
        padw = NCOL - (c1 - c0)

        def shard(v):      # [..., cols] -> padded [... , NCOL]
            s = v[..., c0:c1]
            return np.pad(s, [(0, 0)] * (s.ndim - 1) + [(0, padw)], mode="edge")

        # par: [128, PAR_W]
        par = np.zeros((NPART, PAR_W), f32)
        for i, nm in enumerate(PAR_NAMES):
            par[:, i * NL:(i + 1) * NL] = shard(params[nm]).reshape(NPART, NL)
        wsh = np.pad(w4[:, d * GPC:(d + 1) * GPC],
                     [(0, 0), (0, BPAD - GPC)], mode="edge")  # [LENF, 512]
        # basin b = 4p + j'  ->  par[p, W4_OFF + k*4 + j']
        par[:, W4_OFF:] = wsh.reshape(LENF, NPART, 4).transpose(1, 0, 2).reshape(NPART, LENF * 4)

        # forc: [128, S*NPL*NL], slot ((t*NPL)+f)*NL + j
        fstk = np.stack([shard(pl) for pl in planes], axis=1)   # [S, NPL, NCOL]
        forc = fstk.reshape(S * NPL, NPART, NL).transpose(1, 0, 2).reshape(NPART, S * NPL * NL)

        # pic_seq: [128, S*NL] step-major (for post-loop recovery)
        psh = shard(pic)                                       # [S, NCOL]
        pic_seq = psh.reshape(S, NPART, NL).transpose(1, 0, 2).reshape(NPART, SEQ)

        # pre: [128, 3*NL*S]  (j-major: plane*NL*S + j*S + t)
        pstk = np.stack([shard(pl) for pl in pre_planes], axis=0)  # [3, S, NCOL]
        pre = (pstk.reshape(3, S, NPART, NL)
               .transpose(0, 2, 3, 1)                                # [3,128,NL,S]
               .reshape(3, NPART, NL * S)
               .transpose(1, 0, 2).reshape(NPART, 3 * NL * S))

        per_core.append({"par": np.ascontiguousarray(par),
                         "forc": np.ascontiguousarray(forc),
                         "pre": np.ascontiguousarray(pre),
                         "pics": np.ascontiguousarray(pic_seq)})
    return per_core


# --------------------------------------------------------------------------
# custom DVE ops
# --------------------------------------------------------------------------

def _register_custom_ops():
    from concourse import dve_ops
    from concourse.dve_ops import DveOp, OPS
    from concourse.dve_spec import Spec, Src0, Src1, relu, maxx, lower, C2
    from concourse.dve_uop import DveOpSpec

    made = {}

    def reg(name, spec):
        for op in OPS:
            if op.name == name:
                made[name] = op
                return
        shas = {}
        for ver in ("v3", "v4"):
            uops = lower(spec, ver=ver)
            shas[ver] = DveOpSpec(name=name, opcode=0, uops=uops,
                                  rd1_en=True).sha(ver)
        op = DveOp(name, spec, subdim=False, uops_sha=shas)
        OPS.append(op)
        dve_ops.CUSTOM_DVE_SPECS[name] = spec
        dve_ops._SUB_OPCODE_FOR_NAME[name] = dve_ops._CUSTOM_DVE_ROW_BASE + len(OPS) - 1
        made[name] = op

    reg("SUB_RELU_HYDRO", Spec(
        body=relu(Src0 - Src1),
        reference=lambda in0, in1, *a: np.maximum(in0 - in1, 0).astype(np.float32)))
    reg("SUB_MAXI_HYDRO", Spec(
        body=maxx(Src0 - Src1, C2),
        reference=lambda in0, in1, s0=0.0, s1=0.0, imm2=0.0:
            np.maximum(in0 - in1, imm2).astype(np.float32)))
    reg("ADD_RELU_HYDRO", Spec(
        body=relu(Src0 + Src1),
        reference=lambda in0, in1, *a: np.maximum(in0 + in1, 0).astype(np.float32)))
    # G = in0*(1-in1) = in0 - in0*in1   (infiltration minus recharge)
    reg("MULC_HYDRO", Spec(
        body=Src0 - (Src0 * Src1),
        reference=lambda in0, in1, *a: (in0 - in0 * in1).astype(np.float32)))
    return made


# --------------------------------------------------------------------------
# device program
# --------------------------------------------------------------------------

def _build_program():
    import concourse.bacc as bacc
    import concourse.mybir as mybir
    from concourse.tile import TileContext
    from concourse import bass

    ops = _register_custom_ops()
    SUB_RELU = ops["SUB_RELU_HYDRO"]
    SUB_MAXI = ops["SUB_MAXI_HYDRO"]
    ADD_RELU = ops["ADD_RELU_HYDRO"]
    MULC = ops["MULC_HYDRO"]

    dt = mybir.dt.float32
    AF = mybir.ActivationFunctionType
    OP = mybir.AluOpType

    # Force Ln+Exp to resolve to the combined activation-table set (avoids
    # ~1us table reloads when alternating Ln/Exp each step).
    if not getattr(bacc, "_hydro_act_patch", False):
        _orig_gat = bacc.get_activation_tables

        def _gat(arch):
            tabs = dict(_orig_gat(arch))
            EXP, LN = mybir.ActivationFunctionType.Exp, mybir.ActivationFunctionType.Ln
            if any(n == "natural_log_exp_and_others" and EXP in s and LN in s
                   for n, s in tabs.items()):
                for n in tabs:
                    if n != "natural_log_exp_and_others":
                        tabs[n] = tabs[n] - {EXP, LN}
            return tabs

        bacc.get_activation_tables = _gat
        bacc._hydro_act_patch = True

    nc = bacc.Bacc("TRN2", target_bir_lowering=False, debug=False,
                   num_devices=NCORES)

    d_par = nc.dram_tensor("par", [NPART, PAR_W], dt, kind="ExternalInput").ap()
    d_forc = nc.dram_tensor("forc", [NPART, S * NPL * NL], dt, kind="ExternalInput").ap()
    d_pre = nc.dram_tensor("pre", [NPART, 3 * NL * S], dt, kind="ExternalInput").ap()
    d_pics = nc.dram_tensor("pics", [NPART, SEQ], dt, kind="ExternalInput").ap()
    d_out = nc.dram_tensor("r_out", [NPART, 4 * S], dt, kind="ExternalOutput").ap()

    NZ = float(NEARZERO)

    def subrelu(out, a, b):
        nc.vector._custom_dve(SUB_RELU, out=out, in0=a, in1=b)

    def submaxi(out, a, b, imm):
        nc.vector._custom_dve(SUB_MAXI, out=out, in0=a, in1=b, imm2=imm)

    def addrelu(out, a, b):
        nc.vector._custom_dve(ADD_RELU, out=out, in0=a, in1=b)

    def mulc(out, a, b):
        nc.vector._custom_dve(MULC, out=out, in0=a, in1=b)

    with TileContext(nc) as tc:
        with tc.tile_pool(name="persist", bufs=1) as pp:
            par = pp.tile([NPART, PAR_W], dt, name="par", tag="par")
            nc.sync.dma_start(out=par[:, :], in_=d_par)

            def prm(name):
                i = PAR_NAMES.index(name)
                return par[:, i * NL:(i + 1) * NL]

            CWH, BETA, BLNFC, FC, IFC = (prm(n) for n in
                                         ("cwh", "beta", "blnfc", "fc", "ifc"))
            A1, K0C, CP2, PERC = (prm(n) for n in ("a1", "k0c", "cp2", "perc"))
            SMAX, NFE, FS, QMX, HSM = (prm(n) for n in
                                       ("smax", "nfe", "fs", "qmx", "hsm"))

            def prm1(name, j):     # [P,1] per-partition scalar for lane j
                i = PAR_NAMES.index(name)
                return par[:, i * NL + j: i * NL + j + 1]

            # big sequence buffers (step-major: slot t*NL + j)
            REQ = pp.tile([NPART, SEQ], dt, name="REQ", tag="REQ")
            SUZQ = pp.tile([NPART, SEQ1], dt, name="SUZQ", tag="SUZQ")   # +init
            S1Q = pp.tile([NPART, SEQ1], dt, name="S1Q", tag="S1Q")      # +init
            IN = pp.tile([NPART, SEQ], dt, name="IN", tag="IN")
            PICQ = pp.tile([NPART, SEQ], dt, name="PICQ", tag="PICQ")
            nc.sync.dma_start(out=PICQ[:, :], in_=d_pics)

            # states (sp|mw share one tile so one [128,16] relu advances both)
            SPMW = pp.tile([NPART, 2 * NL], dt, name="SPMW", tag="SPMW")
            SM = pp.tile([NPART, NL], dt, name="SM", tag="SM")

            nc.vector.memset(SPMW[:, :], NZ)
            nc.vector.memset(SM[:, :], 0.5)       # scaled soil state sm/fc
            # SUZP state lives in SUZQ slot t (init: NZ - perc)
            nc.vector.tensor_scalar(out=SUZQ[:, 0:NL], in0=PERC, scalar1=-1.0,
                                    scalar2=NZ, op0=OP.mult, op1=OP.add)
            # s1 state lives in S1Q slot t (init: 0.5*smax)
            nc.vector.tensor_copy(out=S1Q[:, 0:NL], in_=HSM)

            # ---------------- pre-pass: s0 scan + IN (per lane j) ----------
            with tc.tile_pool(name="pre", bufs=2) as prep:
                for j in range(NL):
                    eng = nc.vector if j % 2 == 0 else nc.gpsimd
                    cj = prep.tile([NPART, S], dt, name="cj", tag="cj")
                    bj = prep.tile([NPART, S], dt, name="bj", tag="bj")
                    ptj = prep.tile([NPART, S], dt, name="ptj", tag="ptj")
                    s0j = prep.tile([NPART, S], dt, name="s0j", tag="s0j")
                    nc.sync.dma_start(out=cj[:, :], in_=d_pre[:, 0 * NL * S + j * S: 0 * NL * S + (j + 1) * S])
                    nc.sync.dma_start(out=bj[:, :], in_=d_pre[:, 1 * NL * S + j * S: 1 * NL * S + (j + 1) * S])
                    nc.sync.dma_start(out=ptj[:, :], in_=d_pre[:, 2 * NL * S + j * S: 2 * NL * S + (j + 1) * S])
                    # s0' = max(s0 + c_t, b_t)  (TTS is DVE-only on HW)
                    nc.vector.tensor_tensor_scan(out=s0j[:, :], data0=cj[:, :],
                                                 data1=bj[:, :], initial=NZ,
                                                 op0=OP.add, op1=OP.max)
                    INj = IN[:, j::NL]            # [128, S] strided lane view
                    # IN_t = ptot_t + s0_{t-1} - s0_t   (s0_{-1} = NZ)
                    eng.tensor_tensor(out=INj, in0=ptj[:, :], in1=s0j[:, :],
                                      op=OP.subtract)
                    eng.tensor_tensor(out=IN[:, NL + j::NL], in0=IN[:, NL + j::NL],
                                      in1=s0j[:, 0:S - 1], op=OP.add)
                    eng.tensor_scalar(out=IN[:, j:j + 1], in0=IN[:, j:j + 1],
                                      scalar1=NZ, scalar2=None, op0=OP.add)

            # ---------------- main sequential loop -------------------------
            with tc.tile_pool(name="loop", bufs=6) as lp, \
                 tc.tile_pool(name="chunkp", bufs=1) as cp:
                ET = mybir.EngineType
                UH = U // 2 + 1          # 37 steps in first half
                with tc.For_i(0, NITER, 1,
                              hint_engines=(ET.DVE, ET.Activation, ET.SP)) as iv:
                    chunkA = cp.tile([NPART, UH * NPL * NL], dt, name="chunkA", tag="chunkA")
                    chunkB = cp.tile([NPART, (U - UH) * NPL * NL], dt, name="chunkB", tag="chunkB")
                    nc.sync.dma_start(out=chunkA[:, :],
                                      in_=d_forc[:, bass.ds(iv * (U * NPL * NL), UH * NPL * NL)])
                    nc.sync.dma_start(out=chunkB[:, :],
                                      in_=d_forc[:, bass.ds(iv * (U * NPL * NL) + UH * NPL * NL,
                                                            (U - UH) * NPL * NL)])

                    dtt = nc.vector.tensor_tensor
                    dst = nc.vector.scalar_tensor_tensor
                    dts = nc.vector.tensor_scalar
                    ptt = nc.gpsimd.tensor_tensor

                    def tmp(tag):
                        return lp.tile([NPART, NL], dt, name=tag, tag=tag)[:, :]

                    def suz_chain(t):
                        # upper-zone update for step t (lagged: issued during
                        # step t+1 so it fills the ACT-wait bubbles)
                        SUZPp = SUZQ[:, bass.ds(t * NL, NL)]
                        SUZPn = SUZQ[:, bass.ds((t + 1) * NL, NL)]
                        REslot = REQ[:, bass.ds(t * NL, NL)]
                        SUZ2 = tmp("SUZ2"); addrelu(SUZ2, SUZPp, REslot)
                        TA = tmp("TA"); ptt(out=TA, in0=A1, in1=SUZ2, op=OP.mult)
                        TAp = tmp("TAp"); ptt(out=TAp, in0=TA, in1=PERC, op=OP.subtract)
                        TB = tmp("TB"); ptt(out=TB, in0=K0C, in1=TA, op=OP.mult)
                        TBp = tmp("TBp"); ptt(out=TBp, in0=TB, in1=CP2, op=OP.add)
                        dtt(out=SUZPn, in0=TAp, in1=TBp, op=OP.min)

                    for s in range(U):
                        t = iv * U + s

                        def fr(f):    # forcing plane f at step s
                            if s < UH:
                                o = (s * NPL + f) * NL
                                return chunkA[:, o:o + NL]
                            o = ((s - UH) * NPL + f) * NL
                            return chunkB[:, o:o + NL]

                        D0, NRV, RAIN, PETI, PIC, IPC = (fr(i) for i in range(NPL))
                        # planes 0,1 are adjacent: 16-wide (d0|-rv) view
                        if s < UH:
                            DRV = chunkA[:, (s * NPL) * NL:(s * NPL) * NL + 2 * NL]
                        else:
                            DRV = chunkB[:, ((s - UH) * NPL) * NL:((s - UH) * NPL) * NL + 2 * NL]
                        SPv = SPMW[:, 0:NL]
                        MWv = SPMW[:, NL:2 * NL]

                        S1p = S1Q[:, bass.ds(t * NL, NL)]
                        S1n = S1Q[:, bass.ds((t + 1) * NL, NL)]
                        REslot = REQ[:, bass.ds(t * NL, NL)]
                        INt = IN[:, bass.ds(t * NL, NL)]

                        # -- soil head: Ln on ACT first (only needs sm) --
                        LA = tmp("LA"); nc.scalar.activation(out=LA, in_=SM[:, :], func=AF.Ln)

                        # -- lagged upper-zone update for the previous step --
                        if s > 0:
                            suz_chain(t - 1)

                        # -- snow: one [128,16] relu advances sp and mw --
                        SPMWr = lp.tile([NPART, 2 * NL], dt, name="SPMWr", tag="SPMWr")
                        addrelu(SPMWr[:, :], SPMW[:, :], DRV)
                        SP0 = SPMWr[:, 0:NL]
                        MWr = SPMWr[:, NL:2 * NL]
                        MELT = tmp("MELT"); subrelu(MELT, SPv, SP0)
                        Xs = tmp("Xs"); ptt(out=Xs, in0=SP0, in1=MWv, op=OP.add)
                        ptt(out=SPv, in0=Xs, in1=MWr, op=OP.subtract)
                        MWb = tmp("MWb"); ptt(out=MWb, in0=MWr, in1=MELT, op=OP.add)
                        CAPt = tmp("CAPt"); ptt(out=CAPt, in0=CWH, in1=SPv, op=OP.mult)
                        TSs = tmp("TSs"); subrelu(TSs, MWb, CAPt)
                        ptt(out=MWv, in0=MWb, in1=TSs, op=OP.subtract)
                        RT = tmp("RT"); ptt(out=RT, in0=TSs, in1=RAIN, op=OP.add)
                        # [RTI|SM1B] and [GI|SM2] share tiles so the two RE
                        # feeder subtracts batch into one [128,16] op
                        RS16 = lp.tile([NPART, 2 * NL], dt, name="RS16", tag="RS16")
                        GS16 = lp.tile([NPART, 2 * NL], dt, name="GS16", tag="GS16")
                        RTI = RS16[:, 0:NL]
                        ptt(out=RTI, in0=RT, in1=IFC, op=OP.mult)

                        # -- s1 head + soil head into shared Exp tile --
                        TX = lp.tile([NPART, 2 * NL], dt, name="TX", tag="TX")
                        S1A = tmp("S1A"); ptt(out=S1A, in0=S1p, in1=INt, op=OP.add)
                        S1C = tmp("S1C"); dtt(out=S1C, in0=S1A, in1=SMAX, op=OP.min)
                        S1D = tmp("S1D"); dtt(out=S1D, in0=S1C, in1=PIC, op=OP.mult)
                        T3 = tmp("T3"); dtt(out=T3, in0=NFE, in1=S1D, op=OP.mult)
                        ptt(out=TX[:, NL:2 * NL], in0=T3, in1=FS, op=OP.subtract)
                        # scaled soil: T = beta*ln(SMI) straight into TX half
                        dtt(out=TX[:, 0:NL], in0=BETA, in1=LA, op=OP.mult)
                        EX = lp.tile([NPART, 2 * NL], dt, name="EX", tag="EX")
                        nc.scalar.activation(out=EX[:, :], in_=TX[:, :], func=AF.Exp)
                        SW = EX[:, 0:NL]
                        EE = EX[:, NL:2 * NL]

                        # -- scaled soil tail (SMI = sm/fc):
                        #    GI = RTI*(1-SW); SMI1B = SMI + GI; SMI2 = min(.,1)
                        GI = GS16[:, 0:NL]
                        mulc(GI, RTI, SW)
                        SM1B = RS16[:, NL:2 * NL]
                        dtt(out=SM1B, in0=SM[:, :], in1=GI, op=OP.add)
                        SM2 = GS16[:, NL:2 * NL]
                        dts(out=SM2, in0=SM1B, scalar1=1.0,
                            scalar2=None, op0=OP.min)
                        P1 = tmp("P1"); dtt(out=P1, in0=SM2, in1=IPC, op=OP.mult)
                        P2 = tmp("P2"); submaxi(P2, SM2, PETI, NZ)
                        dtt(out=SM[:, :], in0=P1, in1=P2, op=OP.max)

                        # -- off-cycle: RE (scaled) = (RTI-GI) + (SM1B-SM2),
                        # both subtracts in one 16-wide op --
                        RESUB = lp.tile([NPART, 2 * NL], dt, name="RESUB", tag="RESUB")
                        dtt(out=RESUB[:, :], in0=RS16[:, :], in1=GS16[:, :],
                            op=OP.subtract)
                        ptt(out=REslot, in0=RESUB[:, 0:NL], in1=RESUB[:, NL:2 * NL],
                            op=OP.add)

                        # -- s1 tail (off the sm cycle, issued last) --
                        QB0 = tmp("QB0"); dtt(out=QB0, in0=QMX, in1=EE, op=OP.mult)
                        subrelu(S1n, S1D, QB0)

                    # close the lag within this For_i body
                    suz_chain(iv * U + U - 1)

            # ---------------- post-pass recoveries -------------------------
            dtt = nc.vector.tensor_tensor
            dst = nc.vector.scalar_tensor_tensor
            dts = nc.vector.tensor_scalar
            QHQ = REQ   # QH/QE/Q2 accumulator reuses the RE sequence buffer

            with tc.tile_pool(name="post1", bufs=1) as po, \
                 tc.tile_pool(name="postl", bufs=2) as pl:
                # V = SUZP_prev + RE ; SUZ2 = relu(V)  (full-buffer ops)
                V = po.tile([NPART, SEQ], dt, name="V", tag="V")
                SUZ2Q = po.tile([NPART, SEQ], dt, name="SUZ2Q", tag="SUZ2Q")
                SPL0 = (2 * SEQ // 3) // NL * NL
                dtt(out=V[:, 0:SPL0], in0=SUZQ[:, 0:SPL0], in1=REQ[:, 0:SPL0],
                    op=OP.add)
                nc.gpsimd.tensor_tensor(out=V[:, SPL0:SEQ], in0=SUZQ[:, SPL0:SEQ],
                                        in1=REQ[:, SPL0:SEQ], op=OP.add)
                dts(out=SUZ2Q[:, 0:SPL0], in0=V[:, 0:SPL0], scalar1=0.0,
                    scalar2=None, op0=OP.max)
                nc.gpsimd.tensor_scalar(out=SUZ2Q[:, SPL0:SEQ], in0=V[:, SPL0:SEQ],
                                        scalar1=0.0, scalar2=None, op0=OP.max)

                # QH = (SUZ2 - perc) - SUZP_next   (per lane; REQ now free)
                for j in range(NL):
                    dst(out=QHQ[:, j::NL], in0=SUZ2Q[:, j::NL],
                        scalar=prm1("perc", j), in1=SUZQ[:, NL + j::NL],
                        op0=OP.subtract, op1=OP.subtract)

                # pa = (V + perc) - SUZ2  -> contiguous pac per lane, slz scan,
                # q2 = pac + slz_prev - slz ; fold into QHQ
                ZERO = po.tile([NPART, S], dt, name="zero", tag="zero")
                nc.vector.memset(ZERO[:, :], 0.0)
                for j in range(NL):
                    eng = nc.vector if j % 2 == 0 else nc.gpsimd
                    pac = pl.tile([NPART, S], dt, name="pac", tag="pac")
                    k2cj = pl.tile([NPART, S], dt, name="k2cj", tag="k2cj")
                    slzs = pl.tile([NPART, S], dt, name="slzs", tag="slzs")
                    dst(out=pac[:, :], in0=V[:, j::NL], scalar=prm1("perc", j),
                        in1=SUZ2Q[:, j::NL], op0=OP.add, op1=OP.subtract)
                    eng.tensor_scalar(out=k2cj[:, :], in0=ZERO[:, :],
                                      scalar1=prm1("k2c", j), scalar2=None,
                                      op0=OP.add)
                    # slz' = (pa_t + slz) * k2c   (TTS is DVE-only on HW)
                    nc.vector.tensor_tensor_scan(out=slzs[:, :], data0=pac[:, :],
                                                 data1=k2cj[:, :], initial=NZ,
                                                 op0=OP.add, op1=OP.mult)
                    # q2 = pac + slz_prev - slz'
                    eng.tensor_tensor(out=pac[:, :], in0=pac[:, :],
                                      in1=slzs[:, :], op=OP.subtract)
                    eng.tensor_tensor(out=pac[:, 1:S], in0=pac[:, 1:S],
                                      in1=slzs[:, 0:S - 1], op=OP.add)
                    eng.tensor_scalar(out=pac[:, 0:1], in0=pac[:, 0:1],
                                      scalar1=NZ, scalar2=None, op0=OP.add)
                    eng.tensor_tensor(out=QHQ[:, j::NL], in0=QHQ[:, j::NL],
                                      in1=pac[:, :], op=OP.add)

            with tc.tile_pool(name="post2", bufs=1) as po:
                # s1 recovery: S1A = S1N_prev + IN (in place on IN);
                # QSP = relu(S1A - smax); S1C = S1A - QSP; S1D = S1C*PIC;
                # QB = S1D - S1N_next; QHQ += QSP + QB
                QSPQ = po.tile([NPART, SEQ], dt, name="QSPQ", tag="QSPQ")
                SPL = (2 * SEQ // 3) // NL * NL    # DVE:Pool 2:1 free split

                def big(fn_args):
                    op, a, b = fn_args
                    dtt(out=a[:, 0:SPL], in0=a[:, 0:SPL], in1=b[:, 0:SPL], op=op)
                    nc.gpsimd.tensor_tensor(out=a[:, SPL:SEQ], in0=a[:, SPL:SEQ],
                                            in1=b[:, SPL:SEQ], op=op)

                dtt(out=IN[:, 0:SPL], in0=S1Q[:, 0:SPL], in1=IN[:, 0:SPL], op=OP.add)
                nc.gpsimd.tensor_tensor(out=IN[:, SPL:SEQ], in0=S1Q[:, SPL:SEQ],
                                        in1=IN[:, SPL:SEQ], op=OP.add)
                for j in range(NL):
                    eng = nc.vector if j < 4 else nc.gpsimd
                    eng.tensor_scalar(out=QSPQ[:, j::NL], in0=IN[:, j::NL],
                                      scalar1=prm1("smax", j), scalar2=0.0,
                                      op0=OP.subtract, op1=OP.max)
                big((OP.subtract, IN, QSPQ))
                big((OP.mult, IN, PICQ))
                big((OP.subtract, IN, S1Q[:, NL:SEQ1]))
                for j in range(NL):
                    eng = nc.vector if j < 4 else nc.gpsimd
                    eng.tensor_scalar(out=QHQ[:, j::NL], in0=QHQ[:, j::NL],
                                      scalar1=prm1("fc", j), scalar2=None,
                                      op0=OP.mult)
                big((OP.add, QHQ, QSPQ))
                big((OP.add, QHQ, IN))

                # blend over NMUL -> QS [128, 4*QSW] (lane-major, 14 zero pad)
                QS = po.tile([NPART, 4 * QSW], dt, name="QS", tag="QS")
                nc.vector.memset(QS[:, :], 0.0)
                for jp in range(4):
                    dtt(out=QS[:, jp * QSW + QSPAD: jp * QSW + QSW],
                        in0=QHQ[:, 2 * jp::NL], in1=QHQ[:, 2 * jp + 1::NL], op=OP.add)

                # routing: R[jp, t] = sum_k w4[k, jp] * QS[jp, 14 + t - k]
                # taps 0-9 accumulate on DVE into R; taps 10-14 on Pool into R2
                # taps 0-9 accumulate on DVE via stt into R; taps 10-14 on
                # Pool (ts mult into tmp, tt add into R2 -- Pool has no stt)
                R = po.tile([NPART, 4 * S], dt, name="R", tag="R")
                R2 = po.tile([NPART, 4 * S], dt, name="R2", tag="R2")
                nc.vector.memset(R[:, :], 0.0)
                nc.gpsimd.memset(R2[:, :], 0.0)
                KSPL = 11
                for jp in range(4):
                    rj = R[:, jp * S:(jp + 1) * S]
                    rj2 = R2[:, jp * S:(jp + 1) * S]
                    for k in range(LENF):
                        qsh = QS[:, jp * QSW + QSPAD - k: jp * QSW + QSPAD - k + S]
                        wk = par[:, W4_OFF + k * 4 + jp: W4_OFF + k * 4 + jp + 1]
                        if k < KSPL:
                            dst(out=rj, in0=qsh, scalar=wk, in1=rj,
                                op0=OP.mult, op1=OP.add)
                        else:
                            tp = po.tile([NPART, S], dt, name="tp", tag="tp")
                            nc.gpsimd.tensor_scalar(out=tp[:, :], in0=qsh,
                                                    scalar1=wk, scalar2=None,
                                                    op0=OP.mult)
                            nc.gpsimd.tensor_tensor(out=rj2, in0=rj2,
                                                    in1=tp[:, :], op=OP.add)
                dtt(out=R[:, 0:2 * S], in0=R[:, 0:2 * S], in1=R2[:, 0:2 * S],
                    op=OP.add)
                nc.gpsimd.tensor_tensor(out=R[:, 2 * S:4 * S], in0=R[:, 2 * S:4 * S],
                                        in1=R2[:, 2 * S:4 * S], op=OP.add)

                nc.sync.dma_start(out=d_out, in_=R[:, :])

    nc.compile()
    return nc


_PROGRAM = None


def _get_program():
    global _PROGRAM
    if _PROGRAM is None:
        _PROGRAM = _build_program()
    return _PROGRAM


def kernel(x, raw_phy_static, _trace=False):
    from concourse.bass_utils import run_bass_kernel_spmd

    per_core = _host_prepare(x, raw_phy_static)
    nc = _get_program()
    res = run_bass_kernel_spmd(nc, per_core, core_ids=list(range(NCORES)),
                               trace=_trace)
    out = np.empty((S, G), f32)
    for d in range(NCORES):
        r = res.results[d]["r_out"].reshape(NPART, 4, S)   # [p, jp, t]
        # basin b = 4p + jp
        rb = r.transpose(2, 0, 1).reshape(S, NPART * 4)    # [t, b]
        out[:, d * GPC:(d + 1) * GPC] = rb[:, :GPC]
    if _trace:
        return out, res
    return out


# revision 55
# speedup vs baseline: 1.0392x; 1.0392x over previous
"""Trainium2 Bass kernel for nn_BlendHydroV1 (HBV + ExpHydro blend + gamma routing).

v2 strategy
-----------
Shard 4000 basins over 8 cores (500/core, x NMUL=2 -> 1024 padded columns laid
out [128 partitions, 8 lanes]). Main 730-step loop carries only the five
coupled states (sp, mw, sm, suz-perc, s1) with a minimal fused-op schedule
(~34 DVE/Pool ops + 3 ACT per step after interval-analysis dead-code
elimination):

  * snow: 9 ops via MELT = relu(sp - relu(sp+S-RU)) identity
  * soil: 12 ops + Ln/Exp (rech clip dead since sm<=fc; BLNFC shift)
  * upper zone: 6 ops via suz' = min(a*r - perc, a(1-k0)*r + c - perc)
  * exphydro s1: 7 ops + Exp (ET collapses to S1C*(1-pet/smax) host plane)

Everything else (q0/q1/q2/qspill/qb recovery, pa, slz scan, blend, routing)
runs post-loop as large-free-dim ops from three stored state sequences
(RE, SUZP, S1N) plus precomputed IN.
"""
import numpy as np

S, G, NMUL, LENF = 730, 4000, 2, 15
NCORES = 8
GPC = G // NCORES            # basins per core (500)
BPAD = 512                   # padded basins per core
NCOL = BPAD * NMUL           # 1024 columns
NPART = 128
NL = NCOL // NPART           # 8 lanes per partition
U = 73                       # time steps per For_i iteration
NITER = S // U               # 10
NEARZERO = 1e-5
QSPAD = LENF - 1             # 14
QSW = S + QSPAD              # 744
SEQ = S * NL                 # 5840
SEQ1 = (S + 1) * NL

f32 = np.float32
HBV_LB = np.array([1., 50., .05, .01, .001, .2, 0., 0., -2.5, .5, 0., 0.], f32)
HBV_UB = np.array([6., 1000., .9, .5, .2, 1., 10., 100., 2.5, 10., .1, .2], f32)
EXP_LB = np.array([0., 100., 10., 0., 0., -3.], f32)
EXP_UB = np.array([.1, 1500., 50., 5., 3., 0.], f32)

# param lane order in the `par` DRAM tensor (each NL wide)
PAR_NAMES = ["cwh", "beta", "blnfc", "fc", "ifc", "a1", "k0c", "cp2",
             "perc", "smax", "nfe", "fs", "qmx", "k2c", "hsm"]
NPARAM = len(PAR_NAMES)      # 15
W4_OFF = NPARAM * NL         # 120
PAR_W = W4_OFF + LENF * 4    # 180

# in-loop forcing plane order (slot ((t*NPL)+f)*NL + j)
PLANES = ["d0", "rv", "rain", "pet", "pic", "ipc"]
NPL = len(PLANES)


# --------------------------------------------------------------------------
# host-side preparation
# --------------------------------------------------------------------------

def _host_prepare(x, raw_phy_static):
    """Build per-core DRAM arrays. Returns list of dicts (one per core)."""
    x = np.ascontiguousarray(np.asarray(x, f32))
    raw = np.ascontiguousarray(np.asarray(raw_phy_static, f32))

    static = raw[:, :18 * NMUL].reshape(G, 18, NMUL)
    ph = (HBV_LB[None, :, None] + static[:, :12, :]
          * (HBV_UB - HBV_LB)[None, :, None]).astype(f32)
    pe = (EXP_LB[None, :, None] + static[:, 12:, :]
          * (EXP_UB - EXP_LB)[None, :, None]).astype(f32)

    def cols(a):      # [G, NMUL] -> [G*NMUL], col = g*2+m
        return np.ascontiguousarray(a).reshape(-1)

    beta, fc, k0, k1, k2, lp, perc, uzl, tt, cfmax, cfr, cwh = \
        [cols(ph[:, i, :]) for i in range(12)]
    fexp, smax, qmax, df, tmax, tmin = [cols(pe[:, i, :]) for i in range(6)]

    a1 = (f32(1) - k1).astype(f32)
    params = dict(
        cwh=cwh,
        beta=beta,
        blnfc=(beta * np.log(fc)).astype(f32),
        fc=fc,
        ifc=(1.0 / fc.astype(np.float64)).astype(f32),
        a1=a1,
        k0c=(f32(1) - k0).astype(f32),
        # upper zone runs in sm/fc-scaled units: perc and c are pre-divided
        cp2=(((a1 * k0 * uzl - perc).astype(np.float64)
              / fc.astype(np.float64)).astype(f32)),
        perc=(perc.astype(np.float64) / fc.astype(np.float64)).astype(f32),
        smax=smax,
        nfe=fexp.astype(f32),
        fs=(fexp * smax).astype(f32),
        qmx=qmax,
        k2c=(f32(1) - k2).astype(f32),
        hsm=(f32(0.5) * smax).astype(f32),
    )

    P = x[:, :, 0]
    T = x[:, :, 1]
    PET = x[:, :, 2]
    Pc = np.repeat(P, NMUL, axis=1)
    Tc = np.repeat(T, NMUL, axis=1)
    PETc = np.repeat(PET, NMUL, axis=1)

    rain = np.where(Tc >= tt[None, :], Pc, f32(0)).astype(f32)
    sn = (Pc - rain).astype(f32)
    ru = np.maximum((cfmax[None, :] * (Tc - tt[None, :])).astype(f32), f32(0))
    rv = np.maximum(((cfr * cfmax)[None, :] * (tt[None, :] - Tc)).astype(f32),
                    f32(0))
    d0 = (sn - ru).astype(f32)
    pic = (f32(1) - PETc * (1.0 / smax.astype(np.float64)).astype(f32)[None, :]
           ).astype(f32)
    ilpfc_v = (1.0 / (lp.astype(np.float64) * fc.astype(np.float64))).astype(f32)
    ipc = (f32(1) - PETc * ilpfc_v[None, :]).astype(f32)

    ps = np.where(Tc <= tmin[None, :], Pc, f32(0)).astype(f32)
    mc = np.where(Tc > tmax[None, :],
                  (df[None, :] * (Tc - tmax[None, :])).astype(f32), f32(0))
    scan_c = (ps - mc).astype(f32)

    # routing weights (reference _uh_gamma in f32, scaled by 0.25)
    from scipy.special import gammaln
    route = raw[:, 18 * NMUL:]
    a = (route[:, 0] * f32(2.9)).astype(f32)
    b = (route[:, 1] * f32(6.5)).astype(f32)
    aa = (np.maximum(a, f32(0)) + f32(0.1)).astype(f32)
    th = (np.maximum(b, f32(0)) + f32(0.5)).astype(f32)
    tgrid = (np.arange(LENF, dtype=f32) + f32(0.5))
    logw = (-gammaln(aa.astype(np.float64)).astype(f32)[None, :]
            - (aa * np.log(th).astype(f32))[None, :]
            + np.outer(np.log(tgrid).astype(f32), (aa - f32(1)))
            - np.outer(tgrid, (1.0 / th.astype(np.float64)).astype(f32)))
    w = np.exp(logw.astype(f32)).astype(f32)
    w = (w / w.sum(0, keepdims=True)).astype(f32)          # [LENF, G]
    w4 = (w * f32(0.25)).astype(f32)

    ifc_v = (1.0 / fc.astype(np.float64)).astype(f32)
    peti = (PETc * ifc_v[None, :]).astype(f32)
    nrv = (-rv).astype(f32)
    planes = [d0, nrv, rain, peti, pic, ipc]  # in-loop forcing order
    pre_planes = [scan_c, ps, Pc]           # C, B, PTOT (j-major per core)

    per_core = []
    for d in range(NCORES):
        c0, c1 = d * GPC * NMUL, (d + 1) * GPC * NMUL     # 1000 cols
        padw = NCOL - (c1 - c0)

        def shard(v):      # [..., cols] -> padded [... , NCOL]
            s = v[..., c0:c1]
            return np.pad(s, [(0, 0)] * (s.ndim - 1) + [(0, padw)], mode="edge")

        # par: [128, PAR_W]
        par = np.zeros((NPART, PAR_W), f32)
        for i, nm in enumerate(PAR_NAMES):
            par[:, i * NL:(i + 1) * NL] = shard(params[nm]).reshape(NPART, NL)
        wsh = np.pad(w4[:, d * GPC:(d + 1) * GPC],
                     [(0, 0), (0, BPAD - GPC)], mode="edge")  # [LENF, 512]
        # basin b = 4p + j'  ->  par[p, W4_OFF + k*4 + j']
        par[:, W4_OFF:] = wsh.reshape(LENF, NPART, 4).transpose(1, 0, 2).reshape(NPART, LENF * 4)

        # forc: [128, S*NPL*NL], slot ((t*NPL)+f)*NL + j
        fstk = np.stack([shard(pl) for pl in planes], axis=1)   # [S, NPL, NCOL]
        forc = fstk.reshape(S * NPL, NPART, NL).transpose(1, 0, 2).reshape(NPART, S * NPL * NL)

        # pic_seq: [128, S*NL] step-major (for post-loop recovery)
        psh = shard(pic)                                       # [S, NCOL]
        pic_seq = psh.reshape(S, NPART, NL).transpose(1, 0, 2).reshape(NPART, SEQ)

        # pre: [128, 3*NL*S]  (j-major: plane*NL*S + j*S + t)
        pstk = np.stack([shard(pl) for pl in pre_planes], axis=0)  # [3, S, NCOL]
        pre = (pstk.reshape(3, S, NPART, NL)
               .transpose(0, 2, 3, 1)                                # [3,128,NL,S]
               .reshape(3, NPART, NL * S)
               .transpose(1, 0, 2).reshape(NPART, 3 * NL * S))

        per_core.append({"par": np.ascontiguousarray(par),
                         "forc": np.ascontiguousarray(forc),
                         "pre": np.ascontiguousarray(pre),
                         "pics": np.ascontiguousarray(pic_seq)})
    return per_core


# --------------------------------------------------------------------------
# custom DVE ops
# --------------------------------------------------------------------------

def _register_custom_ops():
    from concourse import dve_ops
    from concourse.dve_ops import DveOp, OPS
    from concourse.dve_spec import Spec, Src0, Src1, relu, maxx, lower, C2
    from concourse.dve_uop import DveOpSpec

    made = {}

    def reg(name, spec):
        for op in OPS:
            if op.name == name:
                made[name] = op
                return
        shas = {}
        for ver in ("v3", "v4"):
            uops = lower(spec, ver=ver)
            shas[ver] = DveOpSpec(name=name, opcode=0, uops=uops,
                                  rd1_en=True).sha(ver)
        op = DveOp(name, spec, subdim=False, uops_sha=shas)
        OPS.append(op)
        dve_ops.CUSTOM_DVE_SPECS[name] = spec
        dve_ops._SUB_OPCODE_FOR_NAME[name] = dve_ops._CUSTOM_DVE_ROW_BASE + len(OPS) - 1
        made[name] = op

    reg("SUB_RELU_HYDRO", Spec(
        body=relu(Src0 - Src1),
        reference=lambda in0, in1, *a: np.maximum(in0 - in1, 0).astype(np.float32)))
    reg("SUB_MAXI_HYDRO", Spec(
        body=maxx(Src0 - Src1, C2),
        reference=lambda in0, in1, s0=0.0, s1=0.0, imm2=0.0:
            np.maximum(in0 - in1, imm2).astype(np.float32)))
    reg("ADD_RELU_HYDRO", Spec(
        body=relu(Src0 + Src1),
        reference=lambda in0, in1, *a: np.maximum(in0 + in1, 0).astype(np.float32)))
    # G = in0*(1-in1) = in0 - in0*in1   (infiltration minus recharge)
    reg("MULC_HYDRO", Spec(
        body=Src0 - (Src0 * Src1),
        reference=lambda in0, in1, *a: (in0 - in0 * in1).astype(np.float32)))
    return made


# --------------------------------------------------------------------------
# device program
# --------------------------------------------------------------------------

def _build_program():
    import concourse.bacc as bacc
    import concourse.mybir as mybir
    from concourse.tile import TileContext
    from concourse import bass

    ops = _register_custom_ops()
    SUB_RELU = ops["SUB_RELU_HYDRO"]
    SUB_MAXI = ops["SUB_MAXI_HYDRO"]
    ADD_RELU = ops["ADD_RELU_HYDRO"]
    MULC = ops["MULC_HYDRO"]

    dt = mybir.dt.float32
    AF = mybir.ActivationFunctionType
    OP = mybir.AluOpType

    # Force Ln+Exp to resolve to the combined activation-table set (avoids
    # ~1us table reloads when alternating Ln/Exp each step).
    if not getattr(bacc, "_hydro_act_patch", False):
        _orig_gat = bacc.get_activation_tables

        def _gat(arch):
            tabs = dict(_orig_gat(arch))
            EXP, LN = mybir.ActivationFunctionType.Exp, mybir.ActivationFunctionType.Ln
            if any(n == "natural_log_exp_and_others" and EXP in s and LN in s
                   for n, s in tabs.items()):
                for n in tabs:
                    if n != "natural_log_exp_and_others":
                        tabs[n] = tabs[n] - {EXP, LN}
            return tabs

        bacc.get_activation_tables = _gat
        bacc._hydro_act_patch = True

    nc = bacc.Bacc("TRN2", target_bir_lowering=False, debug=False,
                   num_devices=NCORES)

    d_par = nc.dram_tensor("par", [NPART, PAR_W], dt, kind="ExternalInput").ap()
    d_forc = nc.dram_tensor("forc", [NPART, S * NPL * NL], dt, kind="ExternalInput").ap()
    d_pre = nc.dram_tensor("pre", [NPART, 3 * NL * S], dt, kind="ExternalInput").ap()
    d_pics = nc.dram_tensor("pics", [NPART, SEQ], dt, kind="ExternalInput").ap()
    d_out = nc.dram_tensor("r_out", [NPART, 4 * S], dt, kind="ExternalOutput").ap()

    NZ = float(NEARZERO)

    def subrelu(out, a, b):
        nc.vector._custom_dve(SUB_RELU, out=out, in0=a, in1=b)

    def submaxi(out, a, b, imm):
        nc.vector._custom_dve(SUB_MAXI, out=out, in0=a, in1=b, imm2=imm)

    def addrelu(out, a, b):
        nc.vector._custom_dve(ADD_RELU, out=out, in0=a, in1=b)

    def mulc(out, a, b):
        nc.vector._custom_dve(MULC, out=out, in0=a, in1=b)

    with TileContext(nc) as tc:
        with tc.tile_pool(name="persist", bufs=1) as pp:
            par = pp.tile([NPART, PAR_W], dt, name="par", tag="par")
            nc.sync.dma_start(out=par[:, :], in_=d_par)

            def prm(name):
                i = PAR_NAMES.index(name)
                return par[:, i * NL:(i + 1) * NL]

            CWH, BETA, BLNFC, FC, IFC = (prm(n) for n in
                                         ("cwh", "beta", "blnfc", "fc", "ifc"))
            A1, K0C, CP2, PERC = (prm(n) for n in ("a1", "k0c", "cp2", "perc"))
            SMAX, NFE, FS, QMX, HSM = (prm(n) for n in
                                       ("smax", "nfe", "fs", "qmx", "hsm"))

            def prm1(name, j):     # [P,1] per-partition scalar for lane j
                i = PAR_NAMES.index(name)
                return par[:, i * NL + j: i * NL + j + 1]

            # big sequence buffers (step-major: slot t*NL + j)
            REQ = pp.tile([NPART, SEQ], dt, name="REQ", tag="REQ")
            SUZQ = pp.tile([NPART, SEQ1], dt, name="SUZQ", tag="SUZQ")   # +init
            S1Q = pp.tile([NPART, SEQ1], dt, name="S1Q", tag="S1Q")      # +init
            IN = pp.tile([NPART, SEQ], dt, name="IN", tag="IN")
            PICQ = pp.tile([NPART, SEQ], dt, name="PICQ", tag="PICQ")
            nc.sync.dma_start(out=PICQ[:, :], in_=d_pics)

            # states (sp|mw share one tile so one [128,16] relu advances both)
            SPMW = pp.tile([NPART, 2 * NL], dt, name="SPMW", tag="SPMW")
            SM = pp.tile([NPART, NL], dt, name="SM", tag="SM")

            nc.vector.memset(SPMW[:, :], NZ)
            nc.vector.memset(SM[:, :], 0.5)       # scaled soil state sm/fc
            # SUZP state lives in SUZQ slot t (init: NZ - perc)
            nc.vector.tensor_scalar(out=SUZQ[:, 0:NL], in0=PERC, scalar1=-1.0,
                                    scalar2=NZ, op0=OP.mult, op1=OP.add)
            # s1 state lives in S1Q slot t (init: 0.5*smax)
            nc.vector.tensor_copy(out=S1Q[:, 0:NL], in_=HSM)

            # ---------------- pre-pass: s0 scan + IN (per lane j) ----------
            with tc.tile_pool(name="pre", bufs=2) as prep:
                for j in range(NL):
                    eng = nc.vector if j % 2 == 0 else nc.gpsimd
                    cj = prep.tile([NPART, S], dt, name="cj", tag="cj")
                    bj = prep.tile([NPART, S], dt, name="bj", tag="bj")
                    ptj = prep.tile([NPART, S], dt, name="ptj", tag="ptj")
                    s0j = prep.tile([NPART, S], dt, name="s0j", tag="s0j")
                    nc.sync.dma_start(out=cj[:, :], in_=d_pre[:, 0 * NL * S + j * S: 0 * NL * S + (j + 1) * S])
                    nc.sync.dma_start(out=bj[:, :], in_=d_pre[:, 1 * NL * S + j * S: 1 * NL * S + (j + 1) * S])
                    nc.sync.dma_start(out=ptj[:, :], in_=d_pre[:, 2 * NL * S + j * S: 2 * NL * S + (j + 1) * S])
                    # s0' = max(s0 + c_t, b_t)  (TTS is DVE-only on HW)
                    nc.vector.tensor_tensor_scan(out=s0j[:, :], data0=cj[:, :],
                                                 data1=bj[:, :], initial=NZ,
                                                 op0=OP.add, op1=OP.max)
                    INj = IN[:, j::NL]            # [128, S] strided lane view
                    # IN_t = ptot_t + s0_{t-1} - s0_t   (s0_{-1} = NZ)
                    eng.tensor_tensor(out=INj, in0=ptj[:, :], in1=s0j[:, :],
                                      op=OP.subtract)
                    eng.tensor_tensor(out=IN[:, NL + j::NL], in0=IN[:, NL + j::NL],
                                      in1=s0j[:, 0:S - 1], op=OP.add)
                    eng.tensor_scalar(out=IN[:, j:j + 1], in0=IN[:, j:j + 1],
                                      scalar1=NZ, scalar2=None, op0=OP.add)

            # ---------------- main sequential loop -------------------------
            with tc.tile_pool(name="loop", bufs=6) as lp, \
                 tc.tile_pool(name="chunkp", bufs=1) as cp:
                ET = mybir.EngineType
                UH = U // 2 + 1          # 37 steps in first half
                with tc.For_i(0, NITER, 1,
                              hint_engines=(ET.DVE, ET.Activation, ET.SP)) as iv:
                    chunkA = cp.tile([NPART, UH * NPL * NL], dt, name="chunkA", tag="chunkA")
                    chunkB = cp.tile([NPART, (U - UH) * NPL * NL], dt, name="chunkB", tag="chunkB")
                    nc.sync.dma_start(out=chunkA[:, :],
                                      in_=d_forc[:, bass.ds(iv * (U * NPL * NL), UH * NPL * NL)])
                    nc.sync.dma_start(out=chunkB[:, :],
                                      in_=d_forc[:, bass.ds(iv * (U * NPL * NL) + UH * NPL * NL,
                                                            (U - UH) * NPL * NL)])

                    dtt = nc.vector.tensor_tensor
                    dst = nc.vector.scalar_tensor_tensor
                    dts = nc.vector.tensor_scalar
                    ptt = nc.gpsimd.tensor_tensor

                    def tmp(tag):
                        return lp.tile([NPART, NL], dt, name=tag, tag=tag)[:, :]

                    def suz_chain(t):
                        # upper-zone update for step t (lagged: issued during
                        # step t+1 so it fills the ACT-wait bubbles)
                        SUZPp = SUZQ[:, bass.ds(t * NL, NL)]
                        SUZPn = SUZQ[:, bass.ds((t + 1) * NL, NL)]
                        REslot = REQ[:, bass.ds(t * NL, NL)]
                        SUZ2 = tmp("SUZ2"); addrelu(SUZ2, SUZPp, REslot)
                        TA = tmp("TA"); ptt(out=TA, in0=A1, in1=SUZ2, op=OP.mult)
                        TAp = tmp("TAp"); ptt(out=TAp, in0=TA, in1=PERC, op=OP.subtract)
                        TB = tmp("TB"); ptt(out=TB, in0=K0C, in1=TA, op=OP.mult)
                        TBp = tmp("TBp"); ptt(out=TBp, in0=TB, in1=CP2, op=OP.add)
                        dtt(out=SUZPn, in0=TAp, in1=TBp, op=OP.min)

                    for s in range(U):
                        t = iv * U + s

                        def fr(f):    # forcing plane f at step s
                            if s < UH:
                                o = (s * NPL + f) * NL
                                return chunkA[:, o:o + NL]
                            o = ((s - UH) * NPL + f) * NL
                            return chunkB[:, o:o + NL]

                        D0, NRV, RAIN, PETI, PIC, IPC = (fr(i) for i in range(NPL))
                        # planes 0,1 are adjacent: 16-wide (d0|-rv) view
                        if s < UH:
                            DRV = chunkA[:, (s * NPL) * NL:(s * NPL) * NL + 2 * NL]
                        else:
                            DRV = chunkB[:, ((s - UH) * NPL) * NL:((s - UH) * NPL) * NL + 2 * NL]
                        SPv = SPMW[:, 0:NL]
                        MWv = SPMW[:, NL:2 * NL]

                        S1p = S1Q[:, bass.ds(t * NL, NL)]
                        S1n = S1Q[:, bass.ds((t + 1) * NL, NL)]
                        REslot = REQ[:, bass.ds(t * NL, NL)]
                        INt = IN[:, bass.ds(t * NL, NL)]

                        # -- soil head: Ln on ACT first (only needs sm) --
                        LA = tmp("LA"); nc.scalar.activation(out=LA, in_=SM[:, :], func=AF.Ln)

                        # -- lagged upper-zone update for the previous step --
                        if s > 0:
                            suz_chain(t - 1)

                        # -- snow: one [128,16] relu advances sp and mw --
                        SPMWr = lp.tile([NPART, 2 * NL], dt, name="SPMWr", tag="SPMWr")
                        addrelu(SPMWr[:, :], SPMW[:, :], DRV)
                        SP0 = SPMWr[:, 0:NL]
                        MWr = SPMWr[:, NL:2 * NL]
                        MELT = tmp("MELT"); subrelu(MELT, SPv, SP0)
                        Xs = tmp("Xs"); ptt(out=Xs, in0=SP0, in1=MWv, op=OP.add)
                        ptt(out=SPv, in0=Xs, in1=MWr, op=OP.subtract)
                        MWb = tmp("MWb"); ptt(out=MWb, in0=MWr, in1=MELT, op=OP.add)
                        CAPt = tmp("CAPt"); ptt(out=CAPt, in0=CWH, in1=SPv, op=OP.mult)
                        TSs = tmp("TSs"); subrelu(TSs, MWb, CAPt)
                        ptt(out=MWv, in0=MWb, in1=TSs, op=OP.subtract)
                        RT = tmp("RT"); ptt(out=RT, in0=TSs, in1=RAIN, op=OP.add)
                        RTI = tmp("RTI"); ptt(out=RTI, in0=RT, in1=IFC, op=OP.mult)

                        # -- s1 head + soil head into shared Exp tile --
                        TX = lp.tile([NPART, 2 * NL], dt, name="TX", tag="TX")
                        S1A = tmp("S1A"); ptt(out=S1A, in0=S1p, in1=INt, op=OP.add)
                        S1C = tmp("S1C"); dtt(out=S1C, in0=S1A, in1=SMAX, op=OP.min)
                        S1D = tmp("S1D"); dtt(out=S1D, in0=S1C, in1=PIC, op=OP.mult)
                        T3 = tmp("T3"); dtt(out=T3, in0=NFE, in1=S1D, op=OP.mult)
                        ptt(out=TX[:, NL:2 * NL], in0=T3, in1=FS, op=OP.subtract)
                        # scaled soil: T = beta*ln(SMI) straight into TX half
                        dtt(out=TX[:, 0:NL], in0=BETA, in1=LA, op=OP.mult)
                        EX = lp.tile([NPART, 2 * NL], dt, name="EX", tag="EX")
                        nc.scalar.activation(out=EX[:, :], in_=TX[:, :], func=AF.Exp)
                        SW = EX[:, 0:NL]
                        EE = EX[:, NL:2 * NL]

                        # -- scaled soil tail (SMI = sm/fc):
                        #    GI = RTI*(1-SW); SMI1B = SMI + GI; SMI2 = min(.,1)
                        GI = tmp("GI"); mulc(GI, RTI, SW)
                        SM1B = tmp("SM1B"); dtt(out=SM1B, in0=SM[:, :], in1=GI, op=OP.add)
                        SM2 = tmp("SM2"); dts(out=SM2, in0=SM1B, scalar1=1.0,
                                              scalar2=None, op0=OP.min)
                        P1 = tmp("P1"); dtt(out=P1, in0=SM2, in1=IPC, op=OP.mult)
                        P2 = tmp("P2"); submaxi(P2, SM2, PETI, NZ)
                        dtt(out=SM[:, :], in0=P1, in1=P2, op=OP.max)

                        # -- off-cycle: RE (scaled) = RECHI + EXSI; the upper
                        # zone runs in the same sm/fc units so no unscale here
                        RECHI = tmp("RECHI"); dtt(out=RECHI, in0=RTI, in1=GI, op=OP.subtract)
                        EXSI = tmp("EXSI"); dtt(out=EXSI, in0=SM1B, in1=SM2, op=OP.subtract)
                        ptt(out=REslot, in0=RECHI, in1=EXSI, op=OP.add)

                        # -- s1 tail (off the sm cycle, issued last) --
                        QB0 = tmp("QB0"); dtt(out=QB0, in0=QMX, in1=EE, op=OP.mult)
                        subrelu(S1n, S1D, QB0)

                    # close the lag within this For_i body
                    suz_chain(iv * U + U - 1)

            # ---------------- post-pass recoveries -------------------------
            dtt = nc.vector.tensor_tensor
            dst = nc.vector.scalar_tensor_tensor
            dts = nc.vector.tensor_scalar
            QHQ = REQ   # QH/QE/Q2 accumulator reuses the RE sequence buffer

            with tc.tile_pool(name="post1", bufs=1) as po, \
                 tc.tile_pool(name="postl", bufs=2) as pl:
                # V = SUZP_prev + RE ; SUZ2 = relu(V)  (full-buffer ops)
                V = po.tile([NPART, SEQ], dt, name="V", tag="V")
                SUZ2Q = po.tile([NPART, SEQ], dt, name="SUZ2Q", tag="SUZ2Q")
                SPL0 = (2 * SEQ // 3) // NL * NL
                dtt(out=V[:, 0:SPL0], in0=SUZQ[:, 0:SPL0], in1=REQ[:, 0:SPL0],
                    op=OP.add)
                nc.gpsimd.tensor_tensor(out=V[:, SPL0:SEQ], in0=SUZQ[:, SPL0:SEQ],
                                        in1=REQ[:, SPL0:SEQ], op=OP.add)
                dts(out=SUZ2Q[:, 0:SPL0], in0=V[:, 0:SPL0], scalar1=0.0,
                    scalar2=None, op0=OP.max)
                nc.gpsimd.tensor_scalar(out=SUZ2Q[:, SPL0:SEQ], in0=V[:, SPL0:SEQ],
                                        scalar1=0.0, scalar2=None, op0=OP.max)

                # QH = (SUZ2 - perc) - SUZP_next   (per lane; REQ now free)
                for j in range(NL):
                    dst(out=QHQ[:, j::NL], in0=SUZ2Q[:, j::NL],
                        scalar=prm1("perc", j), in1=SUZQ[:, NL + j::NL],
                        op0=OP.subtract, op1=OP.subtract)

                # pa = (V + perc) - SUZ2  -> contiguous pac per lane, slz scan,
                # q2 = pac + slz_prev - slz ; fold into QHQ
                ZERO = po.tile([NPART, S], dt, name="zero", tag="zero")
                nc.vector.memset(ZERO[:, :], 0.0)
                for j in range(NL):
                    eng = nc.vector if j % 2 == 0 else nc.gpsimd
                    pac = pl.tile([NPART, S], dt, name="pac", tag="pac")
                    k2cj = pl.tile([NPART, S], dt, name="k2cj", tag="k2cj")
                    slzs = pl.tile([NPART, S], dt, name="slzs", tag="slzs")
                    dst(out=pac[:, :], in0=V[:, j::NL], scalar=prm1("perc", j),
                        in1=SUZ2Q[:, j::NL], op0=OP.add, op1=OP.subtract)
                    eng.tensor_scalar(out=k2cj[:, :], in0=ZERO[:, :],
                                      scalar1=prm1("k2c", j), scalar2=None,
                                      op0=OP.add)
                    # slz' = (pa_t + slz) * k2c   (TTS is DVE-only on HW)
                    nc.vector.tensor_tensor_scan(out=slzs[:, :], data0=pac[:, :],
                                                 data1=k2cj[:, :], initial=NZ,
                                                 op0=OP.add, op1=OP.mult)
                    # q2 = pac + slz_prev - slz'
                    eng.tensor_tensor(out=pac[:, :], in0=pac[:, :],
                                      in1=slzs[:, :], op=OP.subtract)
                    eng.tensor_tensor(out=pac[:, 1:S], in0=pac[:, 1:S],
                                      in1=slzs[:, 0:S - 1], op=OP.add)
                    eng.tensor_scalar(out=pac[:, 0:1], in0=pac[:, 0:1],
                                      scalar1=NZ, scalar2=None, op0=OP.add)
                    eng.tensor_tensor(out=QHQ[:, j::NL], in0=QHQ[:, j::NL],
                                      in1=pac[:, :], op=OP.add)

            with tc.tile_pool(name="post2", bufs=1) as po:
                # s1 recovery: S1A = S1N_prev + IN (in place on IN);
                # QSP = relu(S1A - smax); S1C = S1A - QSP; S1D = S1C*PIC;
                # QB = S1D - S1N_next; QHQ += QSP + QB
                QSPQ = po.tile([NPART, SEQ], dt, name="QSPQ", tag="QSPQ")
                SPL = (2 * SEQ // 3) // NL * NL    # DVE:Pool 2:1 free split

                def big(fn_args):
                    op, a, b = fn_args
                    dtt(out=a[:, 0:SPL], in0=a[:, 0:SPL], in1=b[:, 0:SPL], op=op)
                    nc.gpsimd.tensor_tensor(out=a[:, SPL:SEQ], in0=a[:, SPL:SEQ],
                                            in1=b[:, SPL:SEQ], op=op)

                dtt(out=IN[:, 0:SPL], in0=S1Q[:, 0:SPL], in1=IN[:, 0:SPL], op=OP.add)
                nc.gpsimd.tensor_tensor(out=IN[:, SPL:SEQ], in0=S1Q[:, SPL:SEQ],
                                        in1=IN[:, SPL:SEQ], op=OP.add)
                for j in range(NL):
                    eng = nc.vector if j < 4 else nc.gpsimd
                    eng.tensor_scalar(out=QSPQ[:, j::NL], in0=IN[:, j::NL],
                                      scalar1=prm1("smax", j), scalar2=0.0,
                                      op0=OP.subtract, op1=OP.max)
                big((OP.subtract, IN, QSPQ))
                big((OP.mult, IN, PICQ))
                big((OP.subtract, IN, S1Q[:, NL:SEQ1]))
                for j in range(NL):
                    eng = nc.vector if j < 4 else nc.gpsimd
                    eng.tensor_scalar(out=QHQ[:, j::NL], in0=QHQ[:, j::NL],
                                      scalar1=prm1("fc", j), scalar2=None,
                                      op0=OP.mult)
                big((OP.add, QHQ, QSPQ))
                big((OP.add, QHQ, IN))

                # blend over NMUL -> QS [128, 4*QSW] (lane-major, 14 zero pad)
                QS = po.tile([NPART, 4 * QSW], dt, name="QS", tag="QS")
                nc.vector.memset(QS[:, :], 0.0)
                for jp in range(4):
                    dtt(out=QS[:, jp * QSW + QSPAD: jp * QSW + QSW],
                        in0=QHQ[:, 2 * jp::NL], in1=QHQ[:, 2 * jp + 1::NL], op=OP.add)

                # routing: R[jp, t] = sum_k w4[k, jp] * QS[jp, 14 + t - k]
                # taps 0-9 accumulate on DVE into R; taps 10-14 on Pool into R2
                # taps 0-9 accumulate on DVE via stt into R; taps 10-14 on
                # Pool (ts mult into tmp, tt add into R2 -- Pool has no stt)
                R = po.tile([NPART, 4 * S], dt, name="R", tag="R")
                R2 = po.tile([NPART, 4 * S], dt, name="R2", tag="R2")
                nc.vector.memset(R[:, :], 0.0)
                nc.gpsimd.memset(R2[:, :], 0.0)
                KSPL = 11
                for jp in range(4):
                    rj = R[:, jp * S:(jp + 1) * S]
                    rj2 = R2[:, jp * S:(jp + 1) * S]
                    for k in range(LENF):
                        qsh = QS[:, jp * QSW + QSPAD - k: jp * QSW + QSPAD - k + S]
                        wk = par[:, W4_OFF + k * 4 + jp: W4_OFF + k * 4 + jp + 1]
                        if k < KSPL:
                            dst(out=rj, in0=qsh, scalar=wk, in1=rj,
                                op0=OP.mult, op1=OP.add)
                        else:
                            tp = po.tile([NPART, S], dt, name="tp", tag="tp")
                            nc.gpsimd.tensor_scalar(out=tp[:, :], in0=qsh,
                                                    scalar1=wk, scalar2=None,
                                                    op0=OP.mult)
                            nc.gpsimd.tensor_tensor(out=rj2, in0=rj2,
                                                    in1=tp[:, :], op=OP.add)
                dtt(out=R[:, 0:2 * S], in0=R[:, 0:2 * S], in1=R2[:, 0:2 * S],
                    op=OP.add)
                nc.gpsimd.tensor_tensor(out=R[:, 2 * S:4 * S], in0=R[:, 2 * S:4 * S],
                                        in1=R2[:, 2 * S:4 * S], op=OP.add)

                nc.sync.dma_start(out=d_out, in_=R[:, :])

    nc.compile()
    return nc


_PROGRAM = None


def _get_program():
    global _PROGRAM
    if _PROGRAM is None:
        _PROGRAM = _build_program()
    return _PROGRAM


def kernel(x, raw_phy_static, _trace=False):
    from concourse.bass_utils import run_bass_kernel_spmd

    per_core = _host_prepare(x, raw_phy_static)
    nc = _get_program()
    res = run_bass_kernel_spmd(nc, per_core, core_ids=list(range(NCORES)),
                               trace=_trace)
    out = np.empty((S, G), f32)
    for d in range(NCORES):
        r = res.results[d]["r_out"].reshape(NPART, 4, S)   # [p, jp, t]
        # basin b = 4p + jp
        rb = r.transpose(2, 0, 1).reshape(S, NPART * 4)    # [t, b]
        out[:, d * GPC:(d + 1) * GPC] = rb[:, :GPC]
    if _trace:
        return out, res
    return out
